# revision 1
# baseline (speedup 1.0000x reference)
"""Trainium2 Bass kernel for sparse-conv (kernel-map gather-GEMM-scatter).

Math: out[j, d] = sum over points i with out_idx[i]==j of  x[i, :] @ W[k_idx[i], :, d]

Device strategy ("dense k-slot expansion"):
  Each output voxel j owns 8 kernel-offset slots (k = 0..7); each active input
  point occupies exactly slot (j=out_idx[i], k=k_idx[i]) (unique by construction
  for stride-2/kernel-2 sparse conv). Host scatters x rows into a dense
  xgT[k*32+c, j] tensor; then  out.T = Wflat.T @ xgT  is one plain GEMM with
  K=256 contraction — the device does zero gather/scatter, just DMA + TensorE.
  Output voxels are sharded contiguously across the 8 cores (device-local
  output partitions => no collective needed).

Layout per core (S segs, S multiple of 8192):
  in : xgT  [256, S] (two K-halves of 128 partitions), wflat [256, 32]
  out: out_st [S/8192, 128, 2048]; element [b, 32a+d, 512g+t] holds
       out.T[d, seg] with seg = b*8192 + g*2048 + a*512 + t
"""
import sys

if "/opt/trn_rl_repo" not in sys.path:
    sys.path.insert(0, "/opt/trn_rl_repo")

import numpy as np

N_CORES = 8
BLK = 8192          # segs per staging block ( [128, 2048] staging tile )
DT_IN = "float32"   # dtype of xgT/wflat on device ("float32" or "bfloat16")

_prog_cache = {}


def _build_program(S, dt_name):
    import concourse.tile as tile
    from concourse import bacc, mybir

    dt = getattr(mybir.dt, dt_name)
    f32 = mybir.dt.float32
    nc = bacc.Bacc("TRN2", target_bir_lowering=False, debug=False)
    xgT_d = nc.dram_tensor("xgT", [256, S], dt, kind="ExternalInput")
    w_d = nc.dram_tensor("wflat", [256, 32], dt, kind="ExternalInput")
    nb = S // BLK
    out_d = nc.dram_tensor("out_st", [nb, 128, 2048], f32, kind="ExternalOutput")

    with tile.TileContext(nc) as tc:
        with (
            tc.tile_pool(name="w", bufs=1) as wpool,
            tc.tile_pool(name="xin", bufs=3) as xpool,
            tc.tile_pool(name="st", bufs=2) as stpool,
            tc.tile_pool(name="ps", bufs=8, space="PSUM") as pspool,
        ):
            w0 = wpool.tile([128, 32], dt, tag="w0")
            w1 = wpool.tile([128, 32], dt, tag="w1")
            nc.sync.dma_start(w0[:], w_d.ap()[0:128, :])
            nc.sync.dma_start(w1[:], w_d.ap()[128:256, :])

            for b in range(nb):
                staging = stpool.tile([128, 2048], f32)
                for g in range(4):
                    seg0 = b * BLK + g * 2048
                    x0 = xpool.tile([128, 2048], dt, tag="x0")
                    x1 = xpool.tile([128, 2048], dt, tag="x1")
                    nc.sync.dma_start(x0[:], xgT_d.ap()[0:128, seg0:seg0 + 2048])
                    nc.sync.dma_start(x1[:], xgT_d.ap()[128:256, seg0:seg0 + 2048])
                    for a in range(4):
                        ps = pspool.tile([32, 512], f32)
                        nc.tensor.matmul(ps[:], w0[:], x0[:, 512 * a:512 * (a + 1)],
                                         start=True, stop=False)
                        nc.tensor.matmul(ps[:], w1[:], x1[:, 512 * a:512 * (a + 1)],
                                         start=False, stop=True)
                        eng = nc.vector if (a % 2 == 0) else nc.scalar
                        dst = staging[32 * a:32 * (a + 1), 512 * g:512 * (g + 1)]
                        if eng is nc.vector:
                            eng.tensor_copy(dst, ps[:])
                        else:
                            eng.copy(dst, ps[:])
                nc.sync.dma_start(out_d.ap()[b], staging[:])

    nc.compile()
    return nc


def _get_program(S, dt_name):
    key = (S, dt_name)
    if key not in _prog_cache:
        _prog_cache[key] = _build_program(S, dt_name)
    return _prog_cache[key]


def _pack(x, W, k_idx, out_idx, num_out, dt_np):
    """Host-side: scatter x into dense k-slot layout, per-core [256, S] slabs."""
    n = x.shape[0]
    S = -(-num_out // (N_CORES * BLK)) * BLK  # per-core segs, padded
    Stot = N_CORES * S

    xg4 = np.zeros((Stot, 8, 32), dtype=np.float32)
    pairs = out_idx.astype(np.int64) * 8 + k_idx
    if np.unique(pairs).size == n:
        xg4[out_idx, k_idx] = x
    else:  # duplicate (voxel, offset) pairs: accumulate
        np.add.at(xg4, (out_idx, k_idx), x)

    wflat = W.reshape(256, 32).astype(dt_np)
    in_maps = []
    for c in range(N_CORES):
        slab = xg4[c * S:(c + 1) * S].reshape(S, 256).T  # [256, S]
        in_maps.append({
            "xgT": np.ascontiguousarray(slab).astype(dt_np, copy=False),
            "wflat": wflat,
        })
    return in_maps, S


def _decode(results, S, num_out):
    """Per-core out_st [nb,128,2048] -> out [num_out, 32]."""
    outs = []
    for r in results:
        st = r["out_st"]  # [nb, 128, 2048]
        nb = st.shape[0]
        arr = st.reshape(nb, 4, 32, 4, 512)          # [b, a, d, g, t]
        outT = arr.transpose(2, 0, 3, 1, 4).reshape(32, S)  # [d, seg]
        outs.append(outT.T)                           # [S, 32]
    full = np.concatenate(outs, axis=0)
    return np.ascontiguousarray(full[:num_out])


def run(x, W, k_idx, out_idx, num_out, trace=False, dt_name=DT_IN):
    from concourse.bass_utils import run_bass_kernel_spmd

    x = np.asarray(x, dtype=np.float32)
    W = np.asarray(W, dtype=np.float32)
    k_idx = np.asarray(k_idx, dtype=np.int32)
    out_idx = np.asarray(out_idx, dtype=np.int32)
    num_out = int(num_out)

    dt_np = np.float32
    if dt_name == "bfloat16":
        import ml_dtypes
        dt_np = ml_dtypes.bfloat16

    in_maps, S = _pack(x, W, k_idx, out_idx, num_out, dt_np)
    nc = _get_program(S, dt_name)
    res = run_bass_kernel_spmd(nc, in_maps, list(range(N_CORES)), trace=trace)
    out = _decode(res.results, S, num_out)
    return out, res


def kernel(x, W, k_idx, out_idx, num_out):
    out, _ = run(x, W, k_idx, out_idx, num_out, trace=False)
    return out


# revision 3
# speedup vs baseline: 2.1594x; 2.1594x over previous
"""Trainium2 Bass kernel for sparse-conv (kernel-map gather-GEMM-scatter).

Math: out[j, d] = sum over points i with out_idx[i]==j of  x[i, :] @ W[k_idx[i], :, d]

Device strategy ("dense k-slot expansion"):
  Each output voxel j owns 8 kernel-offset slots (k = 0..7); each active input
  point occupies exactly slot (j=out_idx[i], k=k_idx[i]) (unique by construction
  for stride-2/kernel-2 sparse conv). Host scatters x rows into a dense
  xgT[k*32+c, j] tensor; then  out.T = Wflat.T @ xgT  is one plain GEMM with
  K=256 contraction — the device does zero gather/scatter, just DMA + TensorE.
  Output voxels are sharded contiguously across the 8 cores (device-local
  output partitions => no collective needed).

Layout per core (S segs, S multiple of 8192):
  in : xgT  [256, S] (two K-halves of 128 partitions), wflat [256, 32]
  out: out_st [S/8192, 128, 2048]; element [b, 32a+d, 512g+t] holds
       out.T[d, seg] with seg = b*8192 + g*2048 + a*512 + t
"""
import sys

if "/opt/trn_rl_repo" not in sys.path:
    sys.path.insert(0, "/opt/trn_rl_repo")

import numpy as np

N_CORES = 8
BLK = 8192          # segs per staging block ( [128, 2048] staging tile )
DT_IN = "float32"   # dtype of xgT/wflat on device ("float32" or "bfloat16")

_prog_cache = {}


def _build_program(S, dt_name):
    import concourse.tile as tile
    from concourse import bacc, mybir

    dt = getattr(mybir.dt, dt_name)
    f32 = mybir.dt.float32
    nc = bacc.Bacc("TRN2", target_bir_lowering=False, debug=False)
    xgT_d = nc.dram_tensor("xgT", [256, S], dt, kind="ExternalInput")
    w_d = nc.dram_tensor("wflat", [256, 32], dt, kind="ExternalInput")
    nb = S // BLK
    out_d = nc.dram_tensor("out_st", [nb, 128, 2048], f32, kind="ExternalOutput")

    with tile.TileContext(nc) as tc:
        with (
            tc.tile_pool(name="w", bufs=1) as wpool,
            tc.tile_pool(name="xin", bufs=5) as xpool,
            tc.tile_pool(name="st", bufs=2) as stpool,
            tc.tile_pool(name="ps", bufs=8, space="PSUM") as pspool,
        ):
            w0 = wpool.tile([128, 32], dt, tag="w0")
            w1 = wpool.tile([128, 32], dt, tag="w1")
            nc.sync.dma_start(w0[:], w_d.ap()[0:128, :])
            nc.scalar.dma_start(w1[:], w_d.ap()[128:256, :])

            for b in range(nb):
                staging = stpool.tile([128, 2048], f32)
                for g in range(4):
                    seg0 = b * BLK + g * 2048
                    x0 = xpool.tile([128, 2048], dt, tag="x0")
                    x1 = xpool.tile([128, 2048], dt, tag="x1")
                    nc.sync.dma_start(x0[:], xgT_d.ap()[0:128, seg0:seg0 + 2048])
                    nc.scalar.dma_start(x1[:], xgT_d.ap()[128:256, seg0:seg0 + 2048])
                    for a in range(4):
                        ps = pspool.tile([32, 512], f32)
                        nc.tensor.matmul(ps[:], w0[:], x0[:, 512 * a:512 * (a + 1)],
                                         start=True, stop=False)
                        nc.tensor.matmul(ps[:], w1[:], x1[:, 512 * a:512 * (a + 1)],
                                         start=False, stop=True)
                        eng = nc.vector if (a % 2 == 0) else nc.scalar
                        dst = staging[32 * a:32 * (a + 1), 512 * g:512 * (g + 1)]
                        if eng is nc.vector:
                            eng.tensor_copy(dst, ps[:])
                        else:
                            eng.copy(dst, ps[:])
                nc.gpsimd.dma_start(out_d.ap()[b], staging[:])

    nc.compile()
    return nc


def _get_program(S, dt_name):
    key = (S, dt_name)
    if key not in _prog_cache:
        _prog_cache[key] = _build_program(S, dt_name)
    return _prog_cache[key]


def _pack(x, W, k_idx, out_idx, num_out, dt_np):
    """Host-side: scatter x into dense k-slot layout, per-core [256, S] slabs."""
    n = x.shape[0]
    S = -(-num_out // (N_CORES * BLK)) * BLK  # per-core segs, padded
    Stot = N_CORES * S

    xg4 = np.zeros((Stot, 8, 32), dtype=np.float32)
    pairs = out_idx.astype(np.int64) * 8 + k_idx
    if np.unique(pairs).size == n:
        xg4[out_idx, k_idx] = x
    else:  # duplicate (voxel, offset) pairs: accumulate
        np.add.at(xg4, (out_idx, k_idx), x)

    wflat = W.reshape(256, 32).astype(dt_np)
    in_maps = []
    for c in range(N_CORES):
        slab = xg4[c * S:(c + 1) * S].reshape(S, 256).T  # [256, S]
        in_maps.append({
            "xgT": np.ascontiguousarray(slab).astype(dt_np, copy=False),
            "wflat": wflat,
        })
    return in_maps, S


def _decode(results, S, num_out):
    """Per-core out_st [nb,128,2048] -> out [num_out, 32]."""
    outs = []
    for r in results:
        st = r["out_st"]  # [nb, 128, 2048]
        nb = st.shape[0]
        arr = st.reshape(nb, 4, 32, 4, 512)          # [b, a, d, g, t]
        outT = arr.transpose(2, 0, 3, 1, 4).reshape(32, S)  # [d, seg]
        outs.append(outT.T)                           # [S, 32]
    full = np.concatenate(outs, axis=0)
    return np.ascontiguousarray(full[:num_out])


def run(x, W, k_idx, out_idx, num_out, trace=False, dt_name=DT_IN):
    from concourse.bass_utils import run_bass_kernel_spmd

    x = np.asarray(x, dtype=np.float32)
    W = np.asarray(W, dtype=np.float32)
    k_idx = np.asarray(k_idx, dtype=np.int32)
    out_idx = np.asarray(out_idx, dtype=np.int32)
    num_out = int(num_out)

    if dt_name == "bfloat16":
        import ml_dtypes
        dt_np = ml_dtypes.bfloat16
    else:
        dt_np = {"float32": np.float32, "float16": np.float16}[dt_name]

    in_maps, S = _pack(x, W, k_idx, out_idx, num_out, dt_np)
    nc = _get_program(S, dt_name)
    res = run_bass_kernel_spmd(nc, in_maps, list(range(N_CORES)), trace=trace)
    out = _decode(res.results, S, num_out)
    return out, res


def kernel(x, W, k_idx, out_idx, num_out):
    out, _ = run(x, W, k_idx, out_idx, num_out, trace=False)
    return out
